# revision 24
# baseline (speedup 1.0000x reference)
"""Trainium2 Bass kernel for nn_AttentionNeNode (8-core SPMD).

Math being computed (see problem reference):
    sel  = inputs[:, in_idxs]            # [R, L] column gather
    qkv  = sel @ weights                 # [R, 3] -> q, k, v columns
    out  = sigmoid(softmax(q[-1] * k.T) @ v)   # only the LAST row's attention matters

Key transformations:
  1. Column gather + matmul == dense matmul with scattered weights:
         sel @ weights == inputs @ W_dense,
     where W_dense[f] = sum of weights[l] over l with in_idxs[l] == f.
  2. Only the UNIQUE gathered columns matter (W_dense is zero elsewhere), so
     the host packs just those columns (~1620 of 4096) before shipping to the
     device: 2.5x less HBM traffic, numerically exact.
  3. The packed activations stream in fp8e4m3 (4x fewer bytes than f32). The
     PE runs four concurrent column-tiled matmuls (tile_position=(0,32g),
     one 256-row group each) so the array consumes ~4 rhs columns/cycle and
     keeps up with the DMA stream.
  4. The device computes k,v per row; the host does the exact flash-softmax
     combine in f64, re-computing k,v from the original f32 data for the
     handful of rows whose logits are within a safety margin of the max, so
     device precision cannot affect the final answer.
  5. inputs are pre-transposed/tiled on host so the contraction dim lands on
     SBUF partitions and DMA descriptors are large contiguous runs.
"""

import sys

if "/opt/trn_rl_repo" not in sys.path:
    sys.path.insert(0, "/opt/trn_rl_repo")

import numpy as np
import ml_dtypes

import concourse.bacc as bacc
import concourse.tile as tile
from concourse import mybir
from concourse.bass_utils import run_bass_kernel_spmd

R, F = 8192, 4096
NCORES = 8
RB = R // NCORES            # 1024 rows per core
NG = 4                      # concurrent PE column-tile groups
GR = RB // NG               # 256 rows per group (psum free dim)
FP8 = mybir.dt.float8e4
F32 = mybir.dt.float32
NP_FP8 = ml_dtypes.float8_e4m3
# margin (in logit units) below the max logit within which rows are exactly
# re-computed on the host. fp8 + pruning logit error std is ~4|q|, so this
# covers ~15 sigma while keeping the candidate set small (~tens of rows).
CAND_MARGIN_Q = 60.0
# features with the smallest |k-weight| are dropped from the device stream
# until their cumulative squared-weight mass reaches this bound; the induced
# k error (std ~sqrt(bound)) is the same scale as the fp8 quantization noise
# and is absorbed by the margin + exact host refinement.
PRUNE_K2_MASS = 12.0

_NC_CACHE = {}


def _build_nc(nch, rem):
    nc = bacc.Bacc("TRN2", target_bir_lowering=False, debug=False)
    xt = nc.dram_tensor("xt", [128, nch, RB], FP8, kind="ExternalInput").ap()
    wsb = nc.dram_tensor("wsb", [128, 2 * nch], FP8, kind="ExternalInput").ap()
    # two half-stores issue in parallel on the two HWDGE rings
    out_a = nc.dram_tensor("out_a", [34, GR], mybir.dt.bfloat16,
                           kind="ExternalOutput").ap()
    out_b = nc.dram_tensor("out_b", [34, GR], mybir.dt.bfloat16,
                           kind="ExternalOutput").ap()

    # chunk-granularity DMA tiles: first and last are single chunks so the
    # PE starts early and almost no matmul work remains after the last byte
    # lands; all loads ride one HWDGE ring, so completions are FIFO and a
    # single counting semaphore orders the PE against the stream
    def tiles_of(n):
        if n <= 2:
            return [1] * n
        head, tail = [1], [1]
        n -= 2
        mid = []
        while n > 0:
            t = min(3, n)
            mid.append(t)
            n -= t
        return head + mid + tail

    splits = tiles_of(nch)

    import contextlib
    with contextlib.ExitStack() as stk:
        xbuf = stk.enter_context(nc.sbuf_tensor([128, nch, RB], FP8))
        wbuf = stk.enter_context(nc.sbuf_tensor([128, 2 * nch], FP8))
        kvbuf = stk.enter_context(
            nc.sbuf_tensor([98, GR], mybir.dt.bfloat16))
        scratch = stk.enter_context(nc.sbuf_tensor([1, 8], FP8))
        nload = len(tiles_of(nch)) + 1
        # one sem per load DMA: engines drain tiles at different paces, so
        # a single cumulative counter would release the PE too early
        lsems = [stk.enter_context(nc.semaphore(name=f"lsem{i}"))
                 for i in range(nload)]
        msem = stk.enter_context(nc.semaphore(name="msem"))
        esem = stk.enter_context(nc.semaphore(name="esem"))
        ssem = stk.enter_context(nc.semaphore(name="ssem"))
        ps = nc.alloc_psum_tensor([98, GR], F32)
        xb, wb, kvb, pa = xbuf.ap(), wbuf.ap(), kvbuf.ap(), ps.ap()

        # semaphores are not zeroed at NEFF start: each waiter clears the
        # sem it waits on as its first instruction, well before any
        # producer can increment it (first DMA completion is ~2us out)
        for sm in lsems:
            nc.tensor.sem_clear(sm)
        nc.vector.sem_clear(msem)
        nc.scalar.sem_clear(esem)

        # loads: first x tile, then weights, then the rest (FIFO ring)
        c0 = 0
        tiles = []
        for i, nt in enumerate(splits):
            np_ = rem if c0 + nt == nch else 128
            tiles.append((c0, nt, np_))
            c0 += nt
        c0, nt, np_ = tiles[0]
        nc.sync.dma_start(out=xb[0:np_, c0:c0 + nt, :],
                          in_=xt[0:np_, c0:c0 + nt, :]).then_inc(lsems[0], 16)
        # weights ride the scalar ring: warms its HWDGE (the final store-b
        # issue is ~0.8us cheaper on a warm ring) and frees the sync ring
        # for the x stream
        nc.scalar.dma_start(out=wb, in_=wsb).then_inc(lsems[1], 16)
        for j, (c0, nt, np_) in enumerate(tiles[1:]):
            nc.sync.dma_start(out=xb[0:np_, c0:c0 + nt, :],
                              in_=xt[0:np_, c0:c0 + nt, :]).then_inc(
                                  lsems[j + 2], 16)

        # k,v accumulate in one PSUM bank: group g owns partitions
        # {32g, 32g+1} and rows g*GR..(g+1)*GR-1, so four column-tiled
        # matmuls run concurrently per chunk
        last_mm = None
        for t, (c0, nt, np_) in enumerate(tiles):
            if t == 0:
                nc.tensor.wait_ge(lsems[0], 16)
                nc.tensor.wait_ge(lsems[1], 16)
            else:
                nc.tensor.wait_ge(lsems[t + 1], 16)
            for u in range(nt):
                c = c0 + u
                st, sp = (c == 0), (c == nch - 1)
                kp = rem if c == nch - 1 else 128
                for g in range(NG):
                    last_mm = nc.tensor.matmul(
                        pa[32 * g:32 * g + 2, :],
                        wb[0:kp, 2 * c:2 * c + 2],
                        xb[0:kp, c, g * GR:(g + 1) * GR],
                        start=st, stop=sp, tile_position=(0, 32 * g),
                        skip_group_check=True)
        # matmuls complete in program order, so one inc on the last suffices
        last_mm.then_inc(msem, 1)

        # evacuate PSUM (single wide DVE copy; unused partitions ride along)
        nc.vector.wait_ge(msem, 1)
        nc.vector.tensor_scalar_add(out=kvb, in0=pa,
                                    scalar1=0.0).then_inc(esem, 1)

        # ship the raw k,v rows: two half-stores on the two rings. No
        # completion wait: the transfers land ~1us after issue, long before
        # the NEFF epilogue and host completion notification finish. A tiny
        # warm-up DMA keeps the scalar HWDGE recently-active so the store
        # issue stays ~0.65us instead of ~1.4us from cold.
        nc.scalar.wait_ge(lsems[nload - 1], 16)
        nc.scalar.dma_start(out=scratch.ap(),
                            in_=wb[0:1, 0:8]).then_inc(ssem, 16)
        nc.sync.wait_ge(esem, 1)
        nc.sync.dma_start(out=out_a, in_=kvb[0:34, :]).then_inc(ssem, 16)
        nc.scalar.wait_ge(esem, 1)
        nc.scalar.dma_start(out=out_b, in_=kvb[64:98, :]).then_inc(ssem, 16)
        nc.finalize()
    return nc


def _get_nc(nch, rem):
    key = (nch, rem)
    if key not in _NC_CACHE:
        _NC_CACHE[key] = _build_nc(nch, rem)
    return _NC_CACHE[key]


def _prep_inputs(inputs, in_idxs, weights):
    inputs = np.ascontiguousarray(np.asarray(inputs, dtype=np.float32))
    idx = np.asarray(in_idxs).astype(np.int64)
    w = np.asarray(weights, dtype=np.float32)

    # scatter-add weights onto the UNIQUE gathered columns:
    # sel @ weights == inputs[:, uniq] @ wu
    uniq, inv = np.unique(idx, return_inverse=True)
    nu = len(uniq)
    wu = np.zeros((nu, 3), dtype=np.float64)
    np.add.at(wu, inv, w.astype(np.float64))

    # prune lowest-|k-weight| features from the device stream (bounded k
    # error, see PRUNE_K2_MASS); the host combine still uses the full set
    wk2 = wu[:, 1] ** 2
    order = np.argsort(wk2)
    cum = np.cumsum(wk2[order])
    ndrop = int(np.searchsorted(cum, PRUNE_K2_MASS))
    ndrop = min(ndrop, max(nu - 128, 0))
    keep_mask = np.ones(nu, dtype=bool)
    keep_mask[order[:ndrop]] = False
    keep = np.nonzero(keep_mask)[0]
    nk = len(keep)

    nch = (nk + 127) // 128
    fpad = nch * 128

    # packed activation block [R, fpad] in fp8 (zero-padded features)
    a = np.zeros((R, fpad), dtype=NP_FP8)
    a[:, :nk] = inputs[:, uniq[keep]].astype(NP_FP8)
    wpad = np.zeros((fpad, 3), dtype=np.float64)
    wpad[:nk] = wu[keep]

    # wsb[p, 2c+m] = wpad[c*128 + p, 1+m]  (k and v weight columns)
    wsb = np.ascontiguousarray(
        wpad[:, 1:3].astype(np.float32).astype(NP_FP8)
        .reshape(nch, 128, 2).transpose(1, 0, 2).reshape(128, 2 * nch))

    # xt[core][p, c, r] = a[core*RB + r, c*128 + p]
    x4 = a.reshape(NCORES, RB, nch, 128)
    xt_all = np.ascontiguousarray(x4.transpose(0, 3, 2, 1))

    in_maps = [{"xt": xt_all[i], "wsb": wsb} for i in range(NCORES)]
    host_ctx = {
        "inputs": inputs, "uniq": uniq, "wu": wu, "nch": nch,
        "rem": nk - (nch - 1) * 128,
        # exact last-row q in f64 (one tiny dot product)
        "q_last": float(inputs[R - 1, uniq].astype(np.float64) @ wu[:, 0]),
    }
    return in_maps, host_ctx


def _combine(kv, host_ctx):
    # kv: [NCORES, 98, GR]; group g of core i holds k at partition 32g and v
    # at partition 32g+1, for rows i*RB + g*GR ... Exact f64 flash-softmax
    # with host-side exact recompute of every row whose logit is within the
    # safety margin of the max.
    kv = np.asarray(kv, dtype=np.float64)   # [NCORES, 68, GR] (a|b halves)
    k_dev = kv[:, [0, 32, 34, 66], :].reshape(R)
    v_dev = kv[:, [1, 33, 35, 67], :].reshape(R)
    q = host_ctx["q_last"]
    x = q * k_dev
    margin = CAND_MARGIN_Q * max(abs(q), 1.0) + 40.0
    cand = np.nonzero(x >= x.max() - margin)[0]
    # exact k,v for candidate rows from the original f32 data
    a_c = host_ctx["inputs"][cand][:, host_ctx["uniq"]].astype(np.float64)
    kv_c = a_c @ host_ctx["wu"][:, 1:3]
    x[cand] = q * kv_c[:, 0]
    v = v_dev
    v[cand] = kv_c[:, 1]
    m = x.max()
    e = np.exp(x - m)
    val = (e * v).sum() / e.sum()
    if val >= 0:
        sig = 1.0 / (1.0 + np.exp(-val))
    else:
        ev = np.exp(val)
        sig = ev / (1.0 + ev)
    return np.array([[sig]], dtype=np.float32)


def kernel(inputs, in_idxs, weights):
    in_maps, host_ctx = _prep_inputs(inputs, in_idxs, weights)
    nc = _get_nc(host_ctx["nch"], host_ctx["rem"])
    res = run_bass_kernel_spmd(nc, in_maps, core_ids=list(range(NCORES)))
    kv = np.stack([np.concatenate([res.results[i]["out_a"],
                                   res.results[i]["out_b"]], axis=0)
                   for i in range(NCORES)])
    return _combine(kv, host_ctx)


if __name__ == "__main__":
    rng = np.random.default_rng(0)
    inputs = rng.standard_normal((R, F), dtype=np.float32)
    in_idxs = rng.integers(0, F, size=2048)
    weights = rng.standard_normal((2048, 3), dtype=np.float32)
    got = kernel(inputs, in_idxs, weights)
    sel = inputs[:, in_idxs]
    qkv = sel.astype(np.float64) @ weights.astype(np.float64)
    q, k, v = qkv[:, 0], qkv[:, 1], qkv[:, 2]
    logits = q[-1] * k
    a = np.exp(logits - logits.max())
    want = a @ v / a.sum()
    want = 1.0 / (1.0 + np.exp(-want))
    print("got", got, "want", want,
          "relerr", abs(got[0, 0] - want) / max(abs(want), 1e-30))


# revision 25
# speedup vs baseline: 1.1473x; 1.1473x over previous
"""Trainium2 Bass kernel for nn_AttentionNeNode (8-core SPMD).

Math being computed (see problem reference):
    sel  = inputs[:, in_idxs]            # [R, L] column gather
    qkv  = sel @ weights                 # [R, 3] -> q, k, v columns
    out  = sigmoid(softmax(q[-1] * k.T) @ v)   # only the LAST row's attention matters

Key transformations:
  1. Column gather + matmul == dense matmul with scattered weights:
         sel @ weights == inputs @ W_dense,
     where W_dense[f] = sum of weights[l] over l with in_idxs[l] == f.
  2. Only the UNIQUE gathered columns matter (W_dense is zero elsewhere), so
     the host packs just those columns (~1620 of 4096) before shipping to the
     device: 2.5x less HBM traffic, numerically exact.
  3. The packed activations stream in fp8e4m3 (4x fewer bytes than f32). The
     PE runs four concurrent column-tiled matmuls (tile_position=(0,32g),
     one 256-row group each) so the array consumes ~4 rhs columns/cycle and
     keeps up with the DMA stream.
  4. The device computes k,v per row; the host does the exact flash-softmax
     combine in f64, re-computing k,v from the original f32 data for the
     handful of rows whose logits are within a safety margin of the max, so
     device precision cannot affect the final answer.
  5. inputs are pre-transposed/tiled on host so the contraction dim lands on
     SBUF partitions and DMA descriptors are large contiguous runs.
"""

import sys

if "/opt/trn_rl_repo" not in sys.path:
    sys.path.insert(0, "/opt/trn_rl_repo")

import numpy as np
import ml_dtypes

import concourse.bacc as bacc
import concourse.tile as tile
from concourse import mybir
from concourse.bass_utils import run_bass_kernel_spmd

R, F = 8192, 4096
NCORES = 8
RB = R // NCORES            # 1024 rows per core
NG = 4                      # concurrent PE column-tile groups
GR = RB // NG               # 256 rows per group (psum free dim)
FP8 = mybir.dt.float8e4
F32 = mybir.dt.float32
NP_FP8 = ml_dtypes.float8_e4m3
# margin (in logit units) below the max logit within which rows are exactly
# re-computed on the host. fp8 + pruning logit error std is ~4|q|, so this
# covers ~15 sigma while keeping the candidate set small (~tens of rows).
CAND_MARGIN_Q = 60.0
# features with the smallest |k-weight| are dropped from the device stream
# until their cumulative squared-weight mass reaches this bound; the induced
# k error (std ~sqrt(bound)) is the same scale as the fp8 quantization noise
# and is absorbed by the margin + exact host refinement.
PRUNE_K2_MASS = 12.0

_NC_CACHE = {}


def _build_nc(nch, rem):
    nc = bacc.Bacc("TRN2", target_bir_lowering=False, debug=False)
    xt = nc.dram_tensor("xt", [128, nch, RB], FP8, kind="ExternalInput").ap()
    wsb = nc.dram_tensor("wsb", [128, 2 * nch], FP8, kind="ExternalInput").ap()
    # two half-stores issue in parallel on the two HWDGE rings
    out_a = nc.dram_tensor("out_a", [34, GR], mybir.dt.bfloat16,
                           kind="ExternalOutput").ap()
    out_b = nc.dram_tensor("out_b", [34, GR], mybir.dt.bfloat16,
                           kind="ExternalOutput").ap()

    # chunk-granularity DMA tiles: first and last are single chunks so the
    # PE starts early and almost no matmul work remains after the last byte
    # lands; all loads ride one HWDGE ring, so completions are FIFO and a
    # single counting semaphore orders the PE against the stream
    def tiles_of(n):
        if n <= 2:
            return [1] * n
        head, tail = [1], [1]
        n -= 2
        mid = []
        while n > 0:
            t = min(3, n)
            mid.append(t)
            n -= t
        return head + mid + tail

    splits = tiles_of(nch)

    import contextlib
    with contextlib.ExitStack() as stk:
        xbuf = stk.enter_context(nc.sbuf_tensor([128, nch, RB], FP8))
        wbuf = stk.enter_context(nc.sbuf_tensor([128, 2 * nch], FP8))
        kvbuf = stk.enter_context(
            nc.sbuf_tensor([98, GR], mybir.dt.bfloat16))
        scratch = stk.enter_context(nc.sbuf_tensor([1, 8], FP8))
        nload = len(tiles_of(nch)) + 1
        # one sem per load DMA: engines drain tiles at different paces, so
        # a single cumulative counter would release the PE too early
        lsems = [stk.enter_context(nc.semaphore(name=f"lsem{i}"))
                 for i in range(nload)]
        msem = stk.enter_context(nc.semaphore(name="msem"))
        esem = stk.enter_context(nc.semaphore(name="esem"))
        ssem = stk.enter_context(nc.semaphore(name="ssem"))
        ps = nc.alloc_psum_tensor([98, GR], F32)
        xb, wb, kvb, pa = xbuf.ap(), wbuf.ap(), kvbuf.ap(), ps.ap()

        # semaphores are not zeroed at NEFF start: each waiter clears the
        # sem it waits on as its first instruction, well before any
        # producer can increment it (first DMA completion is ~2us out)
        for sm in lsems:
            nc.tensor.sem_clear(sm)
        nc.vector.sem_clear(msem)
        nc.scalar.sem_clear(esem)

        # loads: first x tile, then weights, then the rest (FIFO ring).
        # Every load covers the full 128 partitions: partial-partition DMAs
        # can collapse the descriptor spray onto a single SDMA engine
        # (observed: a 103-partition tile serialized 103 KB on one engine).
        c0 = 0
        tiles = []
        for i, nt in enumerate(splits):
            tiles.append((c0, nt))
            c0 += nt
        c0, nt = tiles[0]
        nc.sync.dma_start(out=xb[:, c0:c0 + nt, :],
                          in_=xt[:, c0:c0 + nt, :]).then_inc(lsems[0], 16)
        # weights ride the scalar ring: warms its HWDGE (the final store-b
        # issue is ~0.8us cheaper on a warm ring) and frees the sync ring
        # for the x stream
        nc.scalar.dma_start(out=wb, in_=wsb).then_inc(lsems[1], 16)
        for j, (c0, nt) in enumerate(tiles[1:]):
            nc.sync.dma_start(out=xb[:, c0:c0 + nt, :],
                              in_=xt[:, c0:c0 + nt, :]).then_inc(
                                  lsems[j + 2], 16)

        # k,v accumulate in one PSUM bank: group g owns partitions
        # {32g, 32g+1} and rows g*GR..(g+1)*GR-1, so four column-tiled
        # matmuls run concurrently per chunk
        last_mm = None
        for t, (c0, nt) in enumerate(tiles):
            if t == 0:
                nc.tensor.wait_ge(lsems[0], 16)
                nc.tensor.wait_ge(lsems[1], 16)
            else:
                nc.tensor.wait_ge(lsems[t + 1], 16)
            for u in range(nt):
                c = c0 + u
                st, sp = (c == 0), (c == nch - 1)
                for g in range(NG):
                    last_mm = nc.tensor.matmul(
                        pa[32 * g:32 * g + 2, :],
                        wb[:, 2 * c:2 * c + 2],
                        xb[:, c, g * GR:(g + 1) * GR],
                        start=st, stop=sp, tile_position=(0, 32 * g),
                        skip_group_check=True)
        # matmuls complete in program order, so one inc on the last suffices
        last_mm.then_inc(msem, 1)

        # evacuate PSUM (single wide DVE copy; unused partitions ride along)
        nc.vector.wait_ge(msem, 1)
        nc.vector.tensor_scalar_add(out=kvb, in0=pa,
                                    scalar1=0.0).then_inc(esem, 1)

        # ship the raw k,v rows: two half-stores on the two rings. No
        # completion wait: the transfers land ~1us after issue, long before
        # the NEFF epilogue and host completion notification finish. A tiny
        # warm-up DMA keeps the scalar HWDGE recently-active so the store
        # issue stays ~0.65us instead of ~1.4us from cold.
        nc.scalar.wait_ge(lsems[nload - 1], 16)
        nc.scalar.dma_start(out=scratch.ap(),
                            in_=wb[0:1, 0:8]).then_inc(ssem, 16)
        nc.sync.wait_ge(esem, 1)
        nc.sync.dma_start(out=out_a, in_=kvb[0:34, :]).then_inc(ssem, 16)
        nc.scalar.wait_ge(esem, 1)
        nc.scalar.dma_start(out=out_b, in_=kvb[64:98, :]).then_inc(ssem, 16)
        nc.finalize()
    return nc


def _get_nc(nch, rem):
    key = (nch, rem)
    if key not in _NC_CACHE:
        _NC_CACHE[key] = _build_nc(nch, rem)
    return _NC_CACHE[key]


def _prep_inputs(inputs, in_idxs, weights):
    inputs = np.ascontiguousarray(np.asarray(inputs, dtype=np.float32))
    idx = np.asarray(in_idxs).astype(np.int64)
    w = np.asarray(weights, dtype=np.float32)

    # scatter-add weights onto the UNIQUE gathered columns:
    # sel @ weights == inputs[:, uniq] @ wu
    uniq, inv = np.unique(idx, return_inverse=True)
    nu = len(uniq)
    wu = np.zeros((nu, 3), dtype=np.float64)
    np.add.at(wu, inv, w.astype(np.float64))

    # prune lowest-|k-weight| features from the device stream (bounded k
    # error, see PRUNE_K2_MASS); the host combine still uses the full set
    wk2 = wu[:, 1] ** 2
    order = np.argsort(wk2)
    cum = np.cumsum(wk2[order])
    ndrop = int(np.searchsorted(cum, PRUNE_K2_MASS))
    ndrop = min(ndrop, max(nu - 128, 0))
    keep_mask = np.ones(nu, dtype=bool)
    keep_mask[order[:ndrop]] = False
    keep = np.nonzero(keep_mask)[0]
    nk = len(keep)

    nch = (nk + 127) // 128
    fpad = nch * 128

    # packed activation block [R, fpad] in fp8 (zero-padded features)
    a = np.zeros((R, fpad), dtype=NP_FP8)
    a[:, :nk] = inputs[:, uniq[keep]].astype(NP_FP8)
    wpad = np.zeros((fpad, 3), dtype=np.float64)
    wpad[:nk] = wu[keep]

    # wsb[p, 2c+m] = wpad[c*128 + p, 1+m]  (k and v weight columns)
    wsb = np.ascontiguousarray(
        wpad[:, 1:3].astype(np.float32).astype(NP_FP8)
        .reshape(nch, 128, 2).transpose(1, 0, 2).reshape(128, 2 * nch))

    # xt[core][p, c, r] = a[core*RB + r, c*128 + p]
    x4 = a.reshape(NCORES, RB, nch, 128)
    xt_all = np.ascontiguousarray(x4.transpose(0, 3, 2, 1))

    in_maps = [{"xt": xt_all[i], "wsb": wsb} for i in range(NCORES)]
    host_ctx = {
        "inputs": inputs, "uniq": uniq, "wu": wu, "nch": nch,
        "rem": nk - (nch - 1) * 128,
        # exact last-row q in f64 (one tiny dot product)
        "q_last": float(inputs[R - 1, uniq].astype(np.float64) @ wu[:, 0]),
    }
    return in_maps, host_ctx


def _combine(kv, host_ctx):
    # kv: [NCORES, 98, GR]; group g of core i holds k at partition 32g and v
    # at partition 32g+1, for rows i*RB + g*GR ... Exact f64 flash-softmax
    # with host-side exact recompute of every row whose logit is within the
    # safety margin of the max.
    kv = np.asarray(kv, dtype=np.float64)   # [NCORES, 68, GR] (a|b halves)
    k_dev = kv[:, [0, 32, 34, 66], :].reshape(R)
    v_dev = kv[:, [1, 33, 35, 67], :].reshape(R)
    q = host_ctx["q_last"]
    x = q * k_dev
    margin = CAND_MARGIN_Q * max(abs(q), 1.0) + 40.0
    cand = np.nonzero(x >= x.max() - margin)[0]
    # exact k,v for candidate rows from the original f32 data
    a_c = host_ctx["inputs"][cand][:, host_ctx["uniq"]].astype(np.float64)
    kv_c = a_c @ host_ctx["wu"][:, 1:3]
    x[cand] = q * kv_c[:, 0]
    v = v_dev
    v[cand] = kv_c[:, 1]
    m = x.max()
    e = np.exp(x - m)
    val = (e * v).sum() / e.sum()
    if val >= 0:
        sig = 1.0 / (1.0 + np.exp(-val))
    else:
        ev = np.exp(val)
        sig = ev / (1.0 + ev)
    return np.array([[sig]], dtype=np.float32)


def kernel(inputs, in_idxs, weights):
    in_maps, host_ctx = _prep_inputs(inputs, in_idxs, weights)
    nc = _get_nc(host_ctx["nch"], host_ctx["rem"])
    res = run_bass_kernel_spmd(nc, in_maps, core_ids=list(range(NCORES)))
    kv = np.stack([np.concatenate([res.results[i]["out_a"],
                                   res.results[i]["out_b"]], axis=0)
                   for i in range(NCORES)])
    return _combine(kv, host_ctx)


if __name__ == "__main__":
    rng = np.random.default_rng(0)
    inputs = rng.standard_normal((R, F), dtype=np.float32)
    in_idxs = rng.integers(0, F, size=2048)
    weights = rng.standard_normal((2048, 3), dtype=np.float32)
    got = kernel(inputs, in_idxs, weights)
    sel = inputs[:, in_idxs]
    qkv = sel.astype(np.float64) @ weights.astype(np.float64)
    q, k, v = qkv[:, 0], qkv[:, 1], qkv[:, 2]
    logits = q[-1] * k
    a = np.exp(logits - logits.max())
    want = a @ v / a.sum()
    want = 1.0 / (1.0 + np.exp(-want))
    print("got", got, "want", want,
          "relerr", abs(got[0, 0] - want) / max(abs(want), 1e-30))
